# revision 1
# baseline (speedup 1.0000x reference)
"""Trainium2 Bass kernel for nn_DialogueSNN (spiking net over vocab 32000).

Strategy (shipped: variant "full_ps2")
--------------------------------------
Layer-1 (embedding lookup, fc1, m1/spk1 recurrence) is 0.1% of FLOPs and
runs on the host in fp32 with exactly the reference's elementwise op
order; the 0/1 spike train ships as packed u8.  The heavy work runs on 8
NeuronCores, sharding the vocabulary (V=32000 padded to 32768, 4096 rows
= 32 tiles of 128 per core):

  - PE: cur2 = spk1 @ W2.T in float32r with a hi/lo 2-split of W2
    (~22 mantissa bits, near-fp32) at 1 cycle/row per pass.  f32r (not
    bf16) everywhere: measured fastest despite the fused ~107ns
    LDWEIGHTS reload per matmul (bf16's FWL advantage never materialized
    through this toolchain).
  - PSUM: TPS=2 tiles/slot x 4 buffers (deep buffering).  This keeps PE
    ~3 slots ahead of the Act drain chain so every cross-engine
    semaphore is pre-satisfied (~900ns fresh-sem cost on this HW);
    2-deep buffering serialized the PE<->Act handoff per slot.
  - Act: drains all PSUM->SBUF (only PE/Act/DVE have PSUM ports; Act is
    otherwise idle).
  - DVE: the whole m2 LIF recurrence, one fused custom op per step
    ([128,1024] per core-step):  m = (m*beta + c) - (m > thr), bit-exact
    vs the reference's op order; plus spike extract (is_gt -> u8) and
    the u8->f32r spk1 widen.  This 1280-step serial chain is the
    critical path; PE/Act/DMA hide completely beneath it.
  - Pool (gpsimd): idle by design - it shares DVE's 2nd SBUF port pair
    under an exclusive lock, so any Pool activity stalls the 2-source
    LIF op.
  - Spikes leave as u8 (no bit packing), staged 4 tokens per buffer,
    one DMA per 4 tokens (keeps DMA-completion WARs off the DVE chain).

Only the final inner-step spike per token is emitted.
"""
import numpy as np

import concourse.bass as bass
import concourse.tile as tile
from concourse import bacc, mybir

# ---------------- problem constants (hardcoded per harness contract) -------
B, S, V, E, H = 32, 64, 32000, 64, 128
T = 20
BETA = np.float32(0.95)
THR = np.float32(1.0)
N_CORES = 8
VPAD = 32768
V_CORE = VPAD // N_CORES          # 4096 vocab rows per core
NTILE = V_CORE // 128             # 32 V-tiles of 128 per core
F = B * V_CORE // 128             # 1024 m2 elements per partition
NCHUNK = 2                        # chunks per token
CSTEP = T // NCHUNK               # 10 steps per chunk
NCOL = CSTEP * B                  # 320 rhs columns per chunk
TPS = 4                           # tiles per PSUM slot (512-aligned)
NSLOT = NTILE // TPS              # 8 slot fills per chunk

NT_D = 26                         # vocab tiles on DVE's LIF
NT_P = NTILE - NT_D               # vocab tiles on Pool's LIF
F_D = NT_D * B                    # 832
F_P = NT_P * B                    # 192

_DT = mybir.dt


# ---------------- custom DVE op: fused LIF step ----------------------------
def _register_lif_op():
    from concourse.dve_ops import DveOp, OPS, CUSTOM_DVE_SPECS, _SUB_OPCODE_FOR_NAME
    from concourse.dve_spec import Spec, Src0, Src1, C0, C1, lower
    from concourse.dve_uop import DveOpSpec

    name = "LIF_STEP_ANT"
    if name in _SUB_OPCODE_FOR_NAME:
        return next(op for op in OPS if op.name == name)
    body = ((Src0 * C0) + Src1) - (Src0 > C1)

    def ref(in0, in1, s0, s1, imm2):
        return (
            ((in0 * np.float32(s0)).astype(np.float32) + in1).astype(np.float32)
            - (in0 > np.float32(s1)).astype(np.float32)
        ).astype(np.float32)

    spec = Spec(body=body, reference=ref)
    row = max(_SUB_OPCODE_FOR_NAME.values()) + 1
    assert row < 0x20
    _SUB_OPCODE_FOR_NAME[name] = row
    shas = {}
    for ver in ("v3", "v4"):
        uops = lower(spec, ver=ver)
        shas[ver] = DveOpSpec(name=name, opcode=row, uops=uops, rd1_en=True).sha(ver)
    op = DveOp(name, spec, subdim=False, uops_sha=shas)
    OPS.append(op)
    CUSTOM_DVE_SPECS[name] = spec
    return op


# ---------------- host-side helpers ----------------------------------------
def _spk1_host(x, embed, W1, b1):
    """Layer-1 spikes, fp32 elementwise exactly like the reference.
    Returns [S, T, B, H] float32 of 0/1."""
    emb = embed[x]                                            # [B, S, E]
    cur1 = (emb.reshape(-1, E).astype(np.float32) @ W1.T.astype(np.float32)).reshape(
        B, S, H
    ) + b1
    cur1 = cur1.astype(np.float32)
    m1 = np.zeros((B, H), np.float32)
    out = np.zeros((S, T, B, H), np.float32)
    for s in range(S):
        c = cur1[:, s, :]
        for t in range(T):
            r1 = (m1 > THR).astype(np.float32)
            m1 = ((BETA * m1 + c) - r1 * THR).astype(np.float32)
            out[s, t] = m1 - THR > 0
    return out


# ---------------- device module --------------------------------------------
def _build(n_tokens=S, reps=1, variant="full"):
    assert n_tokens % 8 == 0
    lif_op = _register_lif_op()
    nc = bacc.Bacc("TRN2", target_bir_lowering=False, debug=False)

    npairs_pad = n_tokens // 2 + 2
    PW = 2 * T * B                          # 1280 u8 cols per token pair
    spk1_d = nc.dram_tensor(
        "spk1b", [128, npairs_pad * PW], _DT.uint8, kind="ExternalInput"
    ).ap()
    w2_d = nc.dram_tensor("w2t", [128, V_CORE], _DT.float32, kind="ExternalInput").ap()
    out_d = nc.dram_tensor(
        "spk_out", [128, n_tokens * F], _DT.uint8, kind="ExternalOutput"
    ).ap()

    ps2 = "ps2" in variant
    tps = 2 if ps2 else TPS
    nslot = NTILE // tps
    with tile.TileContext(nc) as tc:
        with tc.tile_pool(name="persist", bufs=1) as pp, tc.tile_pool(
            name="work", bufs=1
        ) as wp, tc.tile_pool(name="ps", bufs=(4 if ps2 else 2), space="PSUM") as psp:
            # cur2 ring (3 chunk buffers); w2f shares buffer 0 (prologue only)
            cur2 = [
                pp.tile([128, NCOL * NTILE], _DT.float32, tag=f"cur2_{b}",
                        name=f"cur2_{b}")
                for b in range(3)
            ]
            w2f = pp.tile([128, V_CORE], _DT.float32, tag="cur2_0", name="w2f")
            w2hi = pp.tile([128, V_CORE], _DT.float32r, tag="w2hi")
            w2lo = pp.tile([128, V_CORE], _DT.float32r, tag="w2lo")
            m2d = pp.tile([128, F], _DT.float32, tag="m2d")
            nc.sync.dma_start(w2f[:], w2_d)
            # hi/lo float32r split of W2 (device cast == host rne-11)
            nc.vector.tensor_copy(w2hi[:], w2f[:])
            nc.vector.tensor_tensor(
                w2f[:], w2f[:], w2hi[:].bitcast(_DT.float32), mybir.AluOpType.subtract
            )
            nc.vector.tensor_copy(w2lo[:], w2f[:])
            nc.vector.memset(m2d[:], 0.0)
            if variant.startswith("nomm"):
                for cc in cur2:
                    nc.vector.memset(cc[:], 0.0)

            # spk1 pair buffers (A: even pairs, B: odd pairs), f32r 0/1
            spk1A = pp.tile([128, PW], _DT.float32r, tag="spk1A")
            spk1B = pp.tile([128, PW], _DT.float32r, tag="spk1B")
            # spike staging: 4 tokens per buffer, 2 buffers; one out-DMA per
            # 4 tokens keeps the WAR (DMA-read vs next extract-write) several
            # tokens away from the serial DVE chain.
            out_u8 = [
                pp.tile([128, 4 * F], _DT.uint8, tag=f"outu8{b}", name=f"outu8{b}")
                for b in range(2)
            ]

            NBP = 4                                 # pairs per loop body
            RING = [0, 1, 2, 0, 1, 2, 0, 1, 2, 0, 1, 2, 0, 1, 2, 1]

            def unpack_pair(dram_col_expr, buf, pre=""):
                """DMA one pair's u8 spikes and widen into `buf` (f32r) on DVE."""
                pck = wp.tile([128, PW], _DT.uint8, tag=f"pck{pre}", name=f"pck{pre}")
                if dram_col_expr is None:
                    nc.sync.dma_start(pck[:], spk1_d[:, 0:PW])
                else:
                    base, off = dram_col_expr
                    nc.sync.dma_start(
                        pck[:], spk1_d[:, off:][:, bass.ds(base, PW)]
                    )
                if "aup" in variant:
                    # widen on Act: off the DVE critical chain (feeds PE only)
                    nc.scalar.copy(buf[:], pck[:])
                else:
                    nc.vector.tensor_copy(buf[:], pck[:])

            def compute_token(buf, tok01, out_col, unit_base, phase):
                """Both chunks + LIF + spike emit for one token."""
                for c in range(NCHUNK):
                    unit = unit_base + c
                    cc = cur2[RING[unit]]
                    rhs = buf[:, tok01 * (T * B) + c * NCOL:][:, 0:NCOL]
                    sbsrc = buf if "sbdrain" in variant else None
                    for sl in range(nslot if not variant.startswith("nomm") else 0):
                        ps = psp.tile([128, tps * 512], _DT.float32, tag="ps")
                        for t4 in range(tps):
                            tt = sl * tps + t4
                            dst = ps[:, t4 * 512: t4 * 512 + NCOL]
                            nc.tensor.matmul(
                                dst, w2hi[:, tt * 128:(tt + 1) * 128], rhs,
                                start=True, stop=False,
                            )
                            nc.tensor.matmul(
                                dst, w2lo[:, tt * 128:(tt + 1) * 128], rhs,
                                start=False, stop=True,
                            )
                        if variant != "noact":
                            ps_view = ps[:].rearrange("p (t x) -> p t x", t=tps)
                            base = sl * (tps * NCOL)
                            if sbsrc is not None:
                                # probe: same shape copy but from SBUF
                                nc.scalar.copy(
                                    cc[:, base:base + tps * NCOL],
                                    sbsrc[:, 0:tps * NCOL].bitcast(_DT.float32),
                                )
                            elif "31" in variant:
                                nc.scalar.copy(
                                    cc[:, base:base + 3 * NCOL],
                                    ps_view[:, 0:3, 0:NCOL],
                                )
                                nc.vector.tensor_copy(
                                    cc[:, base + 3 * NCOL:base + 4 * NCOL],
                                    ps_view[:, 3, 0:NCOL],
                                )
                            elif "sm" in variant:
                                # t-major cur2: drain scatters so the LIF
                                # reads one contiguous slab per step
                                ccr = cc[:].rearrange(
                                    "p (t tt b) -> p tt t b", t=CSTEP, tt=NTILE
                                )
                                nc.scalar.copy(
                                    ccr[:, sl * tps:(sl + 1) * tps, :, :],
                                    ps_view[:, :, 0:NCOL],
                                )
                            else:
                                nc.scalar.copy(
                                    cc[:, base:base + tps * NCOL],
                                    ps_view[:, :, 0:NCOL],
                                )
                    cview = cc[:].rearrange(
                        "p (tt t b) -> p tt t b", tt=NTILE, t=CSTEP
                    )
                    for t in range(0 if (variant.startswith("nodve")
                                        or variant == "noact") else CSTEP):
                        if variant == "nomm2":
                            hn = NTILE // 2
                            for hh in range(2):
                                nc.vector._custom_dve(
                                    lif_op,
                                    out=m2d[:, hh * (F // 2):(hh + 1) * (F // 2)],
                                    in0=m2d[:, hh * (F // 2):(hh + 1) * (F // 2)],
                                    in1=cview[:, hh * hn:(hh + 1) * hn, t, :],
                                    s0=float(BETA), s1=float(THR),
                                )
                        else:
                            in1 = (cc[:, t * F:(t + 1) * F] if "sm" in variant
                                   else cview[:, :, t, :])
                            nc.vector._custom_dve(
                                lif_op, out=m2d[:], in0=m2d[:], in1=in1,
                                s0=float(BETA), s1=float(THR),
                            )
                # spikes of the last inner step -> u8 (1 = fired)
                ou, slot = out_col
                nc.vector.tensor_scalar(
                    ou[:, slot * F:(slot + 1) * F], m2d[:], float(THR), None,
                    mybir.AluOpType.is_gt,
                )

            def body(j):
                # iteration j handles NBP pairs (2*NBP tokens), alternating
                # buffers A/B with one-pair unpack lookahead.  Spikes stage
                # into out_u8[k//2] (4 tokens each); one dma per half-body.
                jb = j * (2 * NBP * F)
                jp = j * (NBP * PW)
                for k in range(NBP):
                    ou = out_u8[k // 2]
                    buf = spk1A if k % 2 == 0 else spk1B
                    nbuf = spk1B if k % 2 == 0 else spk1A
                    unpack_pair((jp, (k + 1) * PW), nbuf, pre="ab"[k % 2])
                    compute_token(buf, 0, (ou, (2 * k) % 4), 4 * k, 0)
                    compute_token(buf, 1, (ou, (2 * k + 1) % 4), 4 * k + 2, 1)
                    if k % 2 == 1:
                        nc.sync.dma_start(
                            out_d[:, (k - 1) * 2 * F:][:, bass.ds(jb, 4 * F)],
                            ou[:],
                        )

            # prologue: unpack pair 0 -> A
            unpack_pair(None, spk1A, pre="p")

            assert n_tokens % (2 * NBP) == 0
            nit = n_tokens // (2 * NBP)
            if reps == 1:
                with tc.For_i(0, nit, 1) as j:
                    body(j)
            else:
                with tc.For_i(0, reps, 1) as _r:
                    with tc.For_i(0, nit, 1) as j:
                        body(j)

    nc.finalize()
    return nc


# ---------------- cached PJRT runner ----------------------------------------
_NC_CACHE = {}
_RUN_CACHE = {}


def _get_nc(key):
    if key not in _NC_CACHE:
        _NC_CACHE[key] = _build(*key)
    return _NC_CACHE[key]


def _get_runner(key):
    """Build (once) a cached jitted SPMD executor for the module."""
    if key in _RUN_CACHE:
        return _RUN_CACHE[key]
    import jax
    from jax.sharding import Mesh, PartitionSpec
    from jax.experimental.shard_map import shard_map
    from concourse import bass2jax
    from concourse.bass2jax import (
        _bass_exec_p, install_neuronx_cc_hook, partition_id_tensor,
    )

    install_neuronx_cc_hook()
    nc = _get_nc(key)
    assert nc.dbg_addr is None
    pid_name = nc.partition_id_tensor.name if nc.partition_id_tensor else None

    in_names, out_names, out_avals = [], [], []
    for alloc in nc.m.functions[0].allocations:
        if not isinstance(alloc, mybir.MemoryLocationSet):
            continue
        name = alloc.memorylocations[0].name
        if alloc.kind == "ExternalInput":
            if name == pid_name:
                continue
            in_names.append(name)
        elif alloc.kind == "ExternalOutput":
            out_names.append(name)
            out_avals.append(
                jax.core.ShapedArray(tuple(alloc.tensor_shape), mybir.dt.np(alloc.dtype))
            )
    n_params = len(in_names)
    all_names = tuple(in_names + out_names) + ((pid_name,) if pid_name else ())

    def _body(*args):
        operands = list(args)
        if pid_name:
            operands.append(partition_id_tensor())
        outs = _bass_exec_p.bind(
            *operands,
            out_avals=tuple(out_avals),
            in_names=all_names,
            out_names=tuple(out_names),
            lowering_input_output_aliases=(),
            sim_require_finite=True,
            sim_require_nnan=True,
            nc=nc,
        )
        return tuple(outs)

    devices = jax.devices()[:N_CORES]
    assert len(devices) >= N_CORES, f"need {N_CORES} devices, have {len(devices)}"
    mesh = Mesh(np.asarray(devices), ("core",))
    n_outs = len(out_names)
    sharded = jax.jit(
        shard_map(
            _body,
            mesh=mesh,
            in_specs=(PartitionSpec("core"),) * (n_params + n_outs),
            out_specs=(PartitionSpec("core"),) * n_outs,
            check_rep=False,
        ),
        donate_argnums=tuple(range(n_params, n_params + n_outs)),
        keep_unused=True,
    )
    runner = (sharded, in_names, out_names, out_avals)
    _RUN_CACHE[key] = runner
    return runner


def _run_spmd(key, in_maps):
    sharded, in_names, out_names, out_avals = _get_runner(key)
    concat_in = [
        np.concatenate([in_maps[c][n] for c in range(N_CORES)], axis=0)
        for n in in_names
    ]
    zeros = [
        np.zeros((N_CORES * a.shape[0], *a.shape[1:]), a.dtype) for a in out_avals
    ]
    out_arrs = sharded(*concat_in, *zeros)
    return [
        {
            n: np.asarray(out_arrs[j]).reshape(N_CORES, *out_avals[j].shape)[c]
            for j, n in enumerate(out_names)
        }
        for c in range(N_CORES)
    ]


# ---------------- public entry point ----------------------------------------
def kernel(x, embed, W1, b1, W2, b2, _n_tokens=S, _reps=1, _return_raw=False,
           _variant="full_ps2_aup_sm"):
    x = np.asarray(x)
    embed = np.asarray(embed, np.float32)
    W1 = np.asarray(W1, np.float32)
    b1 = np.asarray(b1, np.float32)
    W2 = np.asarray(W2, np.float32)
    b2 = np.asarray(b2, np.float32)

    # host: layer-1 spikes -> uint8 rhs [128, S*T*B] (+lookahead pad)
    spk1 = _spk1_host(x, embed, W1, b1)                    # [S, T, B, H]
    spk1_rhs = np.ascontiguousarray(spk1.reshape(S * T * B, H).T)
    spk1_bits = np.concatenate(
        [spk1_rhs.astype(np.uint8), np.zeros((128, 4 * T * B), np.uint8)], axis=1
    )

    # host: W2 pad + transpose; hi/lo split happens on device
    W2p = np.zeros((VPAD, H), np.float32)
    W2p[:V] = W2
    W2Tp = np.ascontiguousarray(W2p.T)                     # [128, VPAD]

    in_maps = []
    for k in range(N_CORES):
        sl = slice(k * V_CORE, (k + 1) * V_CORE)
        in_maps.append(
            {"spk1b": spk1_bits, "w2t": np.ascontiguousarray(W2Tp[:, sl])}
        )

    key = (_n_tokens, _reps, _variant)
    results = _run_spmd(key, in_maps)
    if _return_raw:
        return results

    out = np.empty((B, S, VPAD), np.float32)
    for k in range(N_CORES):
        o = results[k]["spk_out"].reshape(128, S, NTILE, B)  # [p, s, tau, b]
        out[:, :, k * V_CORE:(k + 1) * V_CORE] = o.transpose(3, 1, 2, 0).reshape(
            B, S, V_CORE
        ).astype(np.float32)
    return np.ascontiguousarray(out[:, :, :V])



# revision 14
# speedup vs baseline: 4.9924x; 4.9924x over previous
"""Trainium2 Bass kernel for nn_DialogueSNN (spiking net over vocab 32000).

Strategy (shipped: variant "full_ps2")
--------------------------------------
Layer-1 (embedding lookup, fc1, m1/spk1 recurrence) is 0.1% of FLOPs and
runs on the host in fp32 with exactly the reference's elementwise op
order; the 0/1 spike train ships as packed u8.  The heavy work runs on 8
NeuronCores, sharding the vocabulary (V=32000 padded to 32768, 4096 rows
= 32 tiles of 128 per core):

  - PE: cur2 = spk1 @ W2.T in float32r with a hi/lo 2-split of W2
    (~22 mantissa bits, near-fp32) at 1 cycle/row per pass.  f32r (not
    bf16) everywhere: measured fastest despite the fused ~107ns
    LDWEIGHTS reload per matmul (bf16's FWL advantage never materialized
    through this toolchain).
  - PSUM: TPS=2 tiles/slot x 4 buffers (deep buffering).  This keeps PE
    ~3 slots ahead of the Act drain chain so every cross-engine
    semaphore is pre-satisfied (~900ns fresh-sem cost on this HW);
    2-deep buffering serialized the PE<->Act handoff per slot.
  - Act: drains all PSUM->SBUF (only PE/Act/DVE have PSUM ports; Act is
    otherwise idle).
  - DVE: the whole m2 LIF recurrence, one fused custom op per step
    ([128,1024] per core-step):  m = (m*beta + c) - (m > thr), bit-exact
    vs the reference's op order; plus spike extract (is_gt -> u8) and
    the u8->f32r spk1 widen.  This 1280-step serial chain is the
    critical path; PE/Act/DMA hide completely beneath it.
  - Pool (gpsimd): idle by design - it shares DVE's 2nd SBUF port pair
    under an exclusive lock, so any Pool activity stalls the 2-source
    LIF op.
  - Spikes leave as u8 (no bit packing), staged 4 tokens per buffer,
    one DMA per 4 tokens (keeps DMA-completion WARs off the DVE chain).

Only the final inner-step spike per token is emitted.
"""
import numpy as np

import concourse.bass as bass
import concourse.tile as tile
from concourse import bacc, mybir

# ---------------- problem constants (hardcoded per harness contract) -------
B, S, V, E, H = 32, 64, 32000, 64, 128
T = 20
BETA = np.float32(0.95)
THR = np.float32(1.0)
N_CORES = 8
VPAD = 32768
V_CORE = VPAD // N_CORES          # 4096 vocab rows per core
NTILE = V_CORE // 128             # 32 V-tiles of 128 per core
F = B * V_CORE // 128             # 1024 m2 elements per partition
NCHUNK = 2                        # chunks per token
CSTEP = T // NCHUNK               # 10 steps per chunk
NCOL = CSTEP * B                  # 320 rhs columns per chunk
TPS = 4                           # tiles per PSUM slot (512-aligned)
NSLOT = NTILE // TPS              # 8 slot fills per chunk

NT_D = 26                         # vocab tiles on DVE's LIF
NT_P = NTILE - NT_D               # vocab tiles on Pool's LIF
F_D = NT_D * B                    # 832
F_P = NT_P * B                    # 192

_DT = mybir.dt


# ---------------- custom DVE op: fused LIF step ----------------------------
def _register_lif_op():
    from concourse.dve_ops import DveOp, OPS, CUSTOM_DVE_SPECS, _SUB_OPCODE_FOR_NAME
    from concourse.dve_spec import Spec, Src0, Src1, C0, C1, lower
    from concourse.dve_uop import DveOpSpec

    name = "LIF_STEP_ANT"
    if name in _SUB_OPCODE_FOR_NAME:
        return next(op for op in OPS if op.name == name)
    body = ((Src0 * C0) + Src1) - (Src0 > C1)

    def ref(in0, in1, s0, s1, imm2):
        return (
            ((in0 * np.float32(s0)).astype(np.float32) + in1).astype(np.float32)
            - (in0 > np.float32(s1)).astype(np.float32)
        ).astype(np.float32)

    spec = Spec(body=body, reference=ref)
    row = max(_SUB_OPCODE_FOR_NAME.values()) + 1
    assert row < 0x20
    _SUB_OPCODE_FOR_NAME[name] = row
    shas = {}
    for ver in ("v3", "v4"):
        uops = lower(spec, ver=ver)
        shas[ver] = DveOpSpec(name=name, opcode=row, uops=uops, rd1_en=True).sha(ver)
    op = DveOp(name, spec, subdim=False, uops_sha=shas)
    OPS.append(op)
    CUSTOM_DVE_SPECS[name] = spec
    return op


# ---------------- host-side helpers ----------------------------------------
def _spk1_host(x, embed, W1, b1):
    """Layer-1 spikes, fp32 elementwise exactly like the reference.
    Returns [S, T, B, H] float32 of 0/1."""
    emb = embed[x]                                            # [B, S, E]
    cur1 = (emb.reshape(-1, E).astype(np.float32) @ W1.T.astype(np.float32)).reshape(
        B, S, H
    ) + b1
    cur1 = cur1.astype(np.float32)
    m1 = np.zeros((B, H), np.float32)
    out = np.zeros((S, T, B, H), np.float32)
    for s in range(S):
        c = cur1[:, s, :]
        for t in range(T):
            r1 = (m1 > THR).astype(np.float32)
            m1 = ((BETA * m1 + c) - r1 * THR).astype(np.float32)
            out[s, t] = m1 - THR > 0
    return out


# ---------------- device module --------------------------------------------
def _build(n_tokens=S, reps=1, variant="full"):
    assert n_tokens % 8 == 0
    lif_op = _register_lif_op()
    nc = bacc.Bacc("TRN2", target_bir_lowering=False, debug=False)

    npairs_pad = n_tokens // 2 + 2
    PW = 2 * T * B                          # 1280 u8 cols per token pair
    spk1_d = nc.dram_tensor(
        "spk1b", [128, npairs_pad * PW], _DT.uint8, kind="ExternalInput"
    ).ap()
    w2_d = nc.dram_tensor("w2t", [128, V_CORE], _DT.float32, kind="ExternalInput").ap()
    out_d = nc.dram_tensor(
        "spk_out", [128, n_tokens * F], _DT.uint8, kind="ExternalOutput"
    ).ap()

    ps2 = "ps2" in variant
    tps = 2 if ps2 else TPS
    nslot = NTILE // tps
    with tile.TileContext(nc) as tc:
        with tc.tile_pool(name="persist", bufs=1) as pp, tc.tile_pool(
            name="work", bufs=1
        ) as wp, tc.tile_pool(name="ps", bufs=(4 if ps2 else 2), space="PSUM") as psp:
            # cur2 ring (3 chunk buffers); w2f shares buffer 0 (prologue only)
            cur2 = [
                pp.tile([128, NCOL * NTILE], _DT.float32, tag=f"cur2_{b}",
                        name=f"cur2_{b}")
                for b in range(3)
            ]
            w2f = pp.tile([128, V_CORE], _DT.float32, tag="cur2_0", name="w2f")
            w2hi = pp.tile([128, V_CORE], _DT.float32r, tag="w2hi")
            w2lo = pp.tile([128, V_CORE], _DT.float32r, tag="w2lo")
            m2d = pp.tile([128, F], _DT.float32, tag="m2d")
            ea = "ea" in variant
            # "ea": spike extract on Act (Sign) off the DVE chain; m2 ping-pongs
            # per token so the extract of token k reads a buffer no LIF touches
            # until token k+2.
            m2e = (pp.tile([128, F], _DT.float32, tag="m2e", name="m2e")
                   if ea else None)
            bias_nthr = (pp.tile([128, 1], _DT.float32, tag="bias_nthr",
                                 name="bias_nthr") if ea else None)
            if bias_nthr is not None:
                nc.vector.memset(bias_nthr[:], -float(THR))
            nc.sync.dma_start(w2f[:], w2_d)
            # hi/lo float32r split of W2 (device cast == host rne-11)
            nc.vector.tensor_copy(w2hi[:], w2f[:])
            nc.vector.tensor_tensor(
                w2f[:], w2f[:], w2hi[:].bitcast(_DT.float32), mybir.AluOpType.subtract
            )
            nc.vector.tensor_copy(w2lo[:], w2f[:])
            nc.vector.memset(m2d[:], 0.0)
            if m2e is not None:
                nc.vector.memset(m2e[:], 0.0)
            if variant.startswith("nomm"):
                for cc in cur2:
                    nc.vector.memset(cc[:], 0.0)

            # spk1 pair buffers (A: even pairs, B: odd pairs), f32r 0/1
            spk1A = pp.tile([128, PW], _DT.float32r, tag="spk1A")
            spk1B = pp.tile([128, PW], _DT.float32r, tag="spk1B")
            # spike staging: 4 tokens per buffer, 2 buffers; one out-DMA per
            # 4 tokens keeps the WAR (DMA-read vs next extract-write) several
            # tokens away from the serial DVE chain.
            out_u8 = [
                pp.tile([128, 4 * F], _DT.uint8, tag=f"outu8{b}", name=f"outu8{b}")
                for b in range(2)
            ]

            NBP = 32 if "b32" in variant else (16 if "b16" in variant else 4)
            NUNIT = 4 * NBP                         # chunk-units per loop body
            RING = [u % 3 for u in range(NUNIT)]
            if NUNIT % 3 == 1:
                RING[-1] = 1    # avoid same-buffer collision with next body's unit 0

            def unpack_pair(dram_col_expr, buf, pre=""):
                """DMA one pair's u8 spikes and widen into `buf` (f32r) on DVE."""
                pck = wp.tile([128, PW], _DT.uint8, tag=f"pck{pre}", name=f"pck{pre}")
                if "nodma" in variant:
                    pass
                elif dram_col_expr is None:
                    nc.sync.dma_start(pck[:], spk1_d[:, 0:PW])
                else:
                    base, off = dram_col_expr
                    nc.sync.dma_start(
                        pck[:], spk1_d[:, off:][:, bass.ds(base, PW)]
                    )
                if "aup" in variant:
                    # widen on Act: off the DVE critical chain (feeds PE only)
                    nc.scalar.copy(buf[:], pck[:])
                else:
                    nc.vector.tensor_copy(buf[:], pck[:])

            pending_ex = []   # deferred "ea" extracts: (mbuf, ou, slot)

            def emit_extract_act(mbuf, ou, slot):
                # sign(m2 - thr) in {-1,0,+1} cast to u8; host keeps u8==1
                nc.scalar.activation(
                    ou[:, slot * F:(slot + 1) * F], mbuf[:],
                    mybir.ActivationFunctionType.Sign, bias=bias_nthr[:, 0:1],
                )

            def compute_token(buf, tok01, out_col, unit_base, phase, tok_idx=0):
                """Both chunks + LIF + spike emit for one token."""
                if ea:
                    mprev = (m2d, m2e)[tok_idx % 2]
                    mcur = (m2e, m2d)[tok_idx % 2]
                else:
                    mprev = mcur = m2d
                for c in range(NCHUNK):
                    unit = unit_base + c
                    cc = cur2[RING[unit]]
                    rhs = buf[:, tok01 * (T * B) + c * NCOL:][:, 0:NCOL]
                    sbsrc = buf if "sbdrain" in variant else None
                    for sl in range(nslot if not variant.startswith("nomm") else 0):
                        ps = psp.tile([128, tps * 512], _DT.float32, tag="ps")
                        for t4 in range(tps):
                            tt = sl * tps + t4
                            dst = ps[:, t4 * 512: t4 * 512 + NCOL]
                            nc.tensor.matmul(
                                dst, w2hi[:, tt * 128:(tt + 1) * 128], rhs,
                                start=True, stop=False,
                            )
                            nc.tensor.matmul(
                                dst, w2lo[:, tt * 128:(tt + 1) * 128], rhs,
                                start=False, stop=True,
                            )
                        if variant != "noact":
                            ps_view = ps[:].rearrange("p (t x) -> p t x", t=tps)
                            base = sl * (tps * NCOL)
                            if sbsrc is not None:
                                # probe: same shape copy but from SBUF
                                nc.scalar.copy(
                                    cc[:, base:base + tps * NCOL],
                                    sbsrc[:, 0:tps * NCOL].bitcast(_DT.float32),
                                )
                            elif "31" in variant:
                                nc.scalar.copy(
                                    cc[:, base:base + 3 * NCOL],
                                    ps_view[:, 0:3, 0:NCOL],
                                )
                                nc.vector.tensor_copy(
                                    cc[:, base + 3 * NCOL:base + 4 * NCOL],
                                    ps_view[:, 3, 0:NCOL],
                                )
                            elif "sm" in variant:
                                # t-major cur2: drain scatters so the LIF
                                # reads one contiguous slab per step
                                ccr = cc[:].rearrange(
                                    "p (t tt b) -> p tt t b", t=CSTEP, tt=NTILE
                                )
                                nc.scalar.copy(
                                    ccr[:, sl * tps:(sl + 1) * tps, :, :],
                                    ps_view[:, :, 0:NCOL],
                                )
                            else:
                                nc.scalar.copy(
                                    cc[:, base:base + tps * NCOL],
                                    ps_view[:, :, 0:NCOL],
                                )
                    cview = cc[:].rearrange(
                        "p (tt t b) -> p tt t b", tt=NTILE, t=CSTEP
                    )
                    for t in range(0 if (variant.startswith("nodve")
                                        or variant == "noact") else CSTEP):
                        if variant == "nomm2":
                            hn = NTILE // 2
                            for hh in range(2):
                                nc.vector._custom_dve(
                                    lif_op,
                                    out=m2d[:, hh * (F // 2):(hh + 1) * (F // 2)],
                                    in0=m2d[:, hh * (F // 2):(hh + 1) * (F // 2)],
                                    in1=cview[:, hh * hn:(hh + 1) * hn, t, :],
                                    s0=float(BETA), s1=float(THR),
                                )
                        else:
                            in1 = (cc[:, t * F:(t + 1) * F] if "sm" in variant
                                   else cview[:, :, t, :])
                            src = mprev if (c == 0 and t == 0) else mcur
                            nc.vector._custom_dve(
                                lif_op, out=mcur[:], in0=src[:], in1=in1,
                                s0=float(BETA), s1=float(THR),
                            )
                # spikes of the last inner step -> u8 (1 = fired)
                ou, slot = out_col
                if ea:
                    # emit the PREVIOUS token's extract here (keeps Act's FIFO
                    # from blocking this token's drains on the DVE chain),
                    # then defer this token's.
                    if pending_ex:
                        emit_extract_act(*pending_ex.pop())
                    pending_ex.append((mcur, ou, slot))
                else:
                    nc.vector.tensor_scalar(
                        ou[:, slot * F:(slot + 1) * F], m2d[:], float(THR), None,
                        mybir.AluOpType.is_gt,
                    )

            def body(j):
                # iteration j handles NBP pairs (2*NBP tokens), alternating
                # buffers A/B with one-pair unpack lookahead.  Spikes stage
                # into out_u8 (4 tokens each); one dma per 4 tokens.  With
                # "ea" the extract of token t is emitted during token t+1 (so
                # Act's FIFO never gates the DVE chain) and the out-dma slides
                # one token later to stay behind its producers.
                jb = j * (2 * NBP * F)
                jp = j * (NBP * PW)
                for k in range(NBP):
                    ou = out_u8[(k // 2) % 2]
                    buf = spk1A if k % 2 == 0 else spk1B
                    nbuf = spk1B if k % 2 == 0 else spk1A
                    unpack_pair((jp, (k + 1) * PW), nbuf, pre="ab"[k % 2])
                    compute_token(buf, 0, (ou, (2 * k) % 4), 4 * k, 0,
                                  tok_idx=2 * k)
                    if ea and k % 2 == 0 and k > 0 and "nodma" not in variant:
                        oprev = out_u8[(k // 2 + 1) % 2]
                        nc.sync.dma_start(
                            out_d[:, (2 * k - 4) * F:][:, bass.ds(jb, 4 * F)],
                            oprev[:],
                        )
                    compute_token(buf, 1, (ou, (2 * k + 1) % 4), 4 * k + 2, 1,
                                  tok_idx=2 * k + 1)
                    if (not ea) and k % 2 == 1 and "nodma" not in variant:
                        nc.sync.dma_start(
                            out_d[:, (k - 1) * 2 * F:][:, bass.ds(jb, 4 * F)],
                            ou[:],
                        )
                if ea:
                    if pending_ex:
                        emit_extract_act(*pending_ex.pop())
                    if "nodma" not in variant:
                        nc.sync.dma_start(
                            out_d[:, (2 * NBP - 4) * F:][:, bass.ds(jb, 4 * F)],
                            out_u8[((NBP - 1) // 2) % 2][:],
                        )

            # prologue: unpack pair 0 -> A
            unpack_pair(None, spk1A, pre="p")

            assert n_tokens % (2 * NBP) == 0
            nit = n_tokens // (2 * NBP)
            if reps == 1:
                with tc.For_i(0, nit, 1) as j:
                    body(j)
            else:
                with tc.For_i(0, reps, 1) as _r:
                    with tc.For_i(0, nit, 1) as j:
                        body(j)

    nc.finalize()
    return nc


# ---------------- cached PJRT runner ----------------------------------------
_NC_CACHE = {}
_RUN_CACHE = {}


def _get_nc(key):
    if key not in _NC_CACHE:
        _NC_CACHE[key] = _build(*key)
    return _NC_CACHE[key]


def _get_runner(key):
    """Build (once) a cached jitted SPMD executor for the module."""
    if key in _RUN_CACHE:
        return _RUN_CACHE[key]
    import jax
    from jax.sharding import Mesh, PartitionSpec
    from jax.experimental.shard_map import shard_map
    from concourse import bass2jax
    from concourse.bass2jax import (
        _bass_exec_p, install_neuronx_cc_hook, partition_id_tensor,
    )

    install_neuronx_cc_hook()
    nc = _get_nc(key)
    assert nc.dbg_addr is None
    pid_name = nc.partition_id_tensor.name if nc.partition_id_tensor else None

    in_names, out_names, out_avals = [], [], []
    for alloc in nc.m.functions[0].allocations:
        if not isinstance(alloc, mybir.MemoryLocationSet):
            continue
        name = alloc.memorylocations[0].name
        if alloc.kind == "ExternalInput":
            if name == pid_name:
                continue
            in_names.append(name)
        elif alloc.kind == "ExternalOutput":
            out_names.append(name)
            out_avals.append(
                jax.core.ShapedArray(tuple(alloc.tensor_shape), mybir.dt.np(alloc.dtype))
            )
    n_params = len(in_names)
    all_names = tuple(in_names + out_names) + ((pid_name,) if pid_name else ())

    def _body(*args):
        operands = list(args)
        if pid_name:
            operands.append(partition_id_tensor())
        outs = _bass_exec_p.bind(
            *operands,
            out_avals=tuple(out_avals),
            in_names=all_names,
            out_names=tuple(out_names),
            lowering_input_output_aliases=(),
            sim_require_finite=True,
            sim_require_nnan=True,
            nc=nc,
        )
        return tuple(outs)

    devices = jax.devices()[:N_CORES]
    assert len(devices) >= N_CORES, f"need {N_CORES} devices, have {len(devices)}"
    mesh = Mesh(np.asarray(devices), ("core",))
    n_outs = len(out_names)
    sharded = jax.jit(
        shard_map(
            _body,
            mesh=mesh,
            in_specs=(PartitionSpec("core"),) * (n_params + n_outs),
            out_specs=(PartitionSpec("core"),) * n_outs,
            check_rep=False,
        ),
        donate_argnums=tuple(range(n_params, n_params + n_outs)),
        keep_unused=True,
    )
    runner = (sharded, in_names, out_names, out_avals)
    _RUN_CACHE[key] = runner
    return runner


def _run_spmd(key, in_maps):
    sharded, in_names, out_names, out_avals = _get_runner(key)
    concat_in = [
        np.concatenate([in_maps[c][n] for c in range(N_CORES)], axis=0)
        for n in in_names
    ]
    zeros = [
        np.zeros((N_CORES * a.shape[0], *a.shape[1:]), a.dtype) for a in out_avals
    ]
    out_arrs = sharded(*concat_in, *zeros)
    return [
        {
            n: np.asarray(out_arrs[j]).reshape(N_CORES, *out_avals[j].shape)[c]
            for j, n in enumerate(out_names)
        }
        for c in range(N_CORES)
    ]


# ---------------- public entry point ----------------------------------------
def kernel(x, embed, W1, b1, W2, b2, _n_tokens=S, _reps=1, _return_raw=False,
           _variant="full_ps2_aup_sm_ea_b16"):
    x = np.asarray(x)
    embed = np.asarray(embed, np.float32)
    W1 = np.asarray(W1, np.float32)
    b1 = np.asarray(b1, np.float32)
    W2 = np.asarray(W2, np.float32)
    b2 = np.asarray(b2, np.float32)

    # host: layer-1 spikes -> uint8 rhs [128, S*T*B] (+lookahead pad)
    spk1 = _spk1_host(x, embed, W1, b1)                    # [S, T, B, H]
    spk1_rhs = np.ascontiguousarray(spk1.reshape(S * T * B, H).T)
    spk1_bits = np.concatenate(
        [spk1_rhs.astype(np.uint8), np.zeros((128, 4 * T * B), np.uint8)], axis=1
    )

    # host: W2 pad + transpose; hi/lo split happens on device
    W2p = np.zeros((VPAD, H), np.float32)
    W2p[:V] = W2
    W2Tp = np.ascontiguousarray(W2p.T)                     # [128, VPAD]

    in_maps = []
    for k in range(N_CORES):
        sl = slice(k * V_CORE, (k + 1) * V_CORE)
        in_maps.append(
            {"spk1b": spk1_bits, "w2t": np.ascontiguousarray(W2Tp[:, sl])}
        )

    key = (_n_tokens, _reps, _variant)
    results = _run_spmd(key, in_maps)
    if _return_raw:
        return results

    out = np.empty((B, S, VPAD), np.float32)
    for k in range(N_CORES):
        o = results[k]["spk_out"].reshape(128, S, NTILE, B)  # [p, s, tau, b]
        # spike == u8 1 exactly: the "ea" extract emits sign(m2-thr) whose
        # -1 may cast to 0 or 255 depending on HW saturation; ==1 covers both
        # and the non-ea is_gt path (0/1) unchanged.
        out[:, :, k * V_CORE:(k + 1) * V_CORE] = (
            (o == 1).transpose(3, 1, 2, 0).reshape(B, S, V_CORE)
        ).astype(np.float32)
    return np.ascontiguousarray(out[:, :, :V])

